# revision 34
# baseline (speedup 1.0000x reference)
"""LocalGraphAttention TRN2 kernel: 8-core SPMD (data-parallel B x head-parallel).

Layout strategy (per core c): b = c//2, heads = 4*(c%2) + [0..3].
Everything kept transposed so the softmax key-reduction is a PE matmul:
  xT (D, G) -> QT/KT stacks (128=4h*32, G) -> S^T = K @ Q^T per head
  (4-head packed via tile_position row tiling, contraction d=32),
  exp on ScalarE (PSUM->SBUF bf16, no rowmax needed: |scores|<4),
  multiplicative 0/1 mask on VectorE (FD=1024 via partition-free
  broadcast of the shared 512-col mask block across the 2-head span),
  P@V + rowsum via col-tiled matmuls accumulating y^T (128=4h*32, G)
  and rowsums in PSUM (double-buffered per query-group), then a
  normalization + out-projection tail that reuses the pv/rs banks
  (indicator broadcast matmul, reciprocal_approx_fast, scale+bias,
  O^T = WoE.T @ yn) so it overlaps the next query-group's attention.
Host gathers: out[b] = (OT_even + OT_odd).T + b_out.
"""
import sys
import numpy as np
import ml_dtypes

sys.path.insert(0, "/opt/trn_rl_repo")

from contextlib import ExitStack

import concourse.bass as bass
import concourse.mybir as mybir
import concourse.tile as tile
from concourse import bacc
from concourse.bass_utils import run_bass_kernel_spmd
import concourse.dve_ops as dops
from concourse.dve_spec import Spec, Src0, Src1, C0, C1, C2, sq as dve_sq

# Custom DVE exp: em ∝ P3(s)^8 * mask, with P3 a monic deg-3 Horner fit of
# exp(y) on y = s*SCALE/8 (rel spread ~2e-3 over the score range; the
# constant factor cancels exactly in the softmax normalization).
# Offloads part of the softmax exp from the saturated ScalarE to VectorE.
_b1 = ((Src0 + C0) * Src0 + C1) * Src0 + C2


def _ref_poly3(in0, in1, s0, s1, imm2):
    x = in0.astype(np.float32)
    return (((x + s0) * x + s1) * x + imm2).astype(np.float32)


EXP_POLY3 = dops.DveOp(
    "EXP_POLY3M_ANT", Spec(body=_b1, reference=_ref_poly3), subdim=False,
    uops_sha={"v3": "5c7cb73d73810334", "v4": "6aaa157e27d4f5b9"})


def _ref_pow8(in0, in1, s0, s1, imm2):
    u = in0.astype(np.float32) * s0
    return (u ** 8 * in1).astype(np.float32)


EXP_POW8 = dops.DveOp(
    "EXP_POW8S_MASK_ANT",
    Spec(body=dve_sq(dve_sq(dve_sq(Src0 * C0))) * Src1, reference=_ref_pow8),
    subdim=False,
    uops_sha={"v3": "27cd9c8766500aea", "v4": "689a5e63c95e4ee9"})


def _register_dve_op(op):
    if op.name not in dops._SUB_OPCODE_FOR_NAME:
        dops.OPS.append(op)
        dops.CUSTOM_DVE_SPECS[op.name] = op.spec
        dops._SUB_OPCODE_FOR_NAME[op.name] = (
            dops._CUSTOM_DVE_ROW_BASE + len(dops.OPS) - 1)


_register_dve_op(EXP_POLY3)
_register_dve_op(EXP_POW8)

# monic deg-3 coefficients in raw-score space and the pow8 rescale
EXP_A = 139.68961777842475
EXP_B = 12472.81414063659
EXP_C = 564137.3193919723
EXP_INVC = 1.772618058804904e-06
# iterations routed to the VectorE exp path
DVE_EXP_ITERS = set()

BF16 = ml_dtypes.bfloat16
G = 2048
D = 256
NH = 8
DH = 32
B = 4
NCORES = 8
SCALE = 1.0 / np.sqrt(np.float32(DH))
KB = G // 128   # 16 key blocks
QG = G // 512   # 4 query groups


def build_nc():
    nc = bacc.Bacc("TRN2", target_bir_lowering=False, debug=False)
    dt = mybir.dt
    xT = nc.declare_dram_parameter("xT", [D, G], dt.bfloat16, isOutput=False)
    Wq = nc.declare_dram_parameter("Wq", [D, 128], dt.bfloat16, isOutput=False)
    Wk = nc.declare_dram_parameter("Wk", [D, 128], dt.bfloat16, isOutput=False)
    Wv = nc.declare_dram_parameter("Wv", [D, 128], dt.bfloat16, isOutput=False)
    bq = nc.declare_dram_parameter("bq", [128, 1], dt.float32, isOutput=False)
    bk = nc.declare_dram_parameter("bk", [128, 1], dt.float32, isOutput=False)
    bv = nc.declare_dram_parameter("bv", [128, 1], dt.float32, isOutput=False)
    M01T = nc.declare_dram_parameter("M01T", [G, G], dt.bfloat16, isOutput=False)
    WoE = nc.declare_dram_parameter("WoE", [128, D], dt.bfloat16, isOutput=False)
    OUT = nc.declare_dram_parameter("out", [D, G], dt.bfloat16, isOutput=True)

    with tile.TileContext(nc) as tc, ExitStack() as ctx:
        singles = ctx.enter_context(tc.tile_pool(name="singles", bufs=1))
        maskp = ctx.enter_context(tc.tile_pool(name="maskp", bufs=KB))
        vp = ctx.enter_context(tc.tile_pool(name="vp", bufs=KB))
        work = ctx.enter_context(tc.tile_pool(name="work", bufs=6))
        tailp = ctx.enter_context(tc.tile_pool(name="tailp", bufs=2))
        psq = ctx.enter_context(tc.tile_pool(name="psq", bufs=3, space="PSUM"))
        ppv = ctx.enter_context(tc.tile_pool(name="ppv", bufs=1, space="PSUM"))

        # ---- resident loads ----
        xt = []
        for kc in range(2):
            t = singles.tile([128, G], dt.bfloat16, tag=f"xt{kc}")
            for ch in range(4):
                csl = slice(512 * ch, 512 * (ch + 1))
                nc.sync.dma_start(out=t[:, csl],
                                  in_=xT[128 * kc:128 * (kc + 1), csl])
            xt.append(t)
        wght = {}
        for name, p in (("wq", Wq), ("wk", Wk), ("wv", Wv)):
            for kc in range(2):
                t = singles.tile([128, 128], dt.bfloat16, tag=f"{name}{kc}")
                nc.sync.dma_start(out=t[:], in_=p[128 * kc:128 * (kc + 1), :])
                wght[f"{name}{kc}"] = t
        bq_sb = singles.tile([128, 1], dt.float32, tag="bq")
        nc.sync.dma_start(out=bq_sb[:], in_=bq[:])
        bk_sb = singles.tile([128, 1], dt.float32, tag="bk")
        nc.sync.dma_start(out=bk_sb[:], in_=bk[:])
        bv_sb = singles.tile([128, 1], dt.float32, tag="bv")
        nc.sync.dma_start(out=bv_sb[:], in_=bv[:])
        woe_sb = singles.tile([128, D], dt.bfloat16, tag="woe")
        nc.sync.dma_start(out=woe_sb[:], in_=WoE[:])
        m_sb = []
        for kb in range(KB):
            t = maskp.tile([128, G], dt.bfloat16, tag="mask")
            for ch in range(2):
                csl = slice(1024 * ch, 1024 * (ch + 1))
                nc.sync.dma_start(out=t[:, csl],
                                  in_=M01T[128 * kb:128 * (kb + 1), csl])
            m_sb.append(t)
        ones_sb = singles.tile([128, 1], dt.bfloat16, tag="ones")
        nc.vector.memset(ones_sb[:], 1.0)

        ind4b = singles.tile([128, 128], dt.bfloat16, tag="ind4b")
        nc.vector.memset(ind4b[:], 0.0)
        for h in range(4):
            nc.vector.memset(ind4b[32 * h:32 * h + 1, 32 * h:32 * (h + 1)],
                             1.0)

        # ---- QKV projections (kt first so scores can start earliest) ----
        qt_sb = singles.tile([128, G], dt.bfloat16, tag="qt")
        kt_sb = singles.tile([128, G], dt.bfloat16, tag="kt")
        for dst, wn, b_sb in ((kt_sb, "wk", bk_sb), (qt_sb, "wq", bq_sb)):
            for qg in range(QG):
                ps = psq.tile([128, 1024], dt.float32, tag="sq")
                sl = slice(512 * qg, 512 * (qg + 1))
                nc.tensor.matmul(ps[:, 0:512], wght[wn + "0"][:],
                                 xt[0][:, sl], start=True, stop=False)
                nc.tensor.matmul(ps[:, 0:512], wght[wn + "1"][:],
                                 xt[1][:, sl], start=False, stop=True)
                nc.vector.tensor_scalar_add(dst[:, sl], ps[:, 0:512], b_sb[:])
        v_sb = []
        for kb in range(KB):
            ps = psq.tile([128, 1024], dt.float32, tag="sq")
            sl = slice(128 * kb, 128 * (kb + 1))
            nc.tensor.matmul(ps[:, 0:128], xt[0][:, sl], wght["wv0"][:],
                             start=True, stop=False)
            nc.tensor.matmul(ps[:, 0:128], xt[1][:, sl], wght["wv1"][:],
                             start=False, stop=True)
            t = vp.tile([128, 128], dt.bfloat16, tag="v")
            nc.vector.tensor_copy(t[:], ps[:, 0:128])
            v_sb.append(t)

        # ---- attention (software-pipelined per-qg tail) ----
        yn_sb = singles.tile([128, G], dt.bfloat16, tag="yn")

        def emit_scores(qg, kb):
            qsl = slice(512 * qg, 512 * (qg + 1))
            sqs = [psq.tile([128, 1024], dt.float32, tag="sq", name=f"sq{qg}_{kb}_{p}")
                   for p in range(2)]
            for pair in range(2):
                for j in range(2):
                    h = 2 * pair + j
                    hsl = slice(32 * h, 32 * (h + 1))
                    nc.tensor.matmul(
                        sqs[pair][:, 512 * j:512 * (j + 1)],
                        kt_sb[hsl, 128 * kb:128 * (kb + 1)],
                        qt_sb[hsl, qsl],
                        start=True, stop=True, tile_position=(32 * h, 0))
            return sqs

        def emit_softmax_pv(qg, kb, sqs, pv_ps, rs_ps):
            qsl = slice(512 * qg, 512 * (qg + 1))
            mb = m_sb[kb][:, qsl].unsqueeze(1).broadcast_to([128, 4, 512])
            em2 = work.tile([128, 2048], dt.bfloat16, tag="em", bufs=4,
                            name=f"em{qg}_{kb}")
            if (qg, kb) in DVE_EXP_ITERS:
                u2 = work.tile([128, 2048], dt.float32, tag="u", bufs=2,
                               name=f"u{qg}_{kb}")
                for pair in range(2):
                    nc.vector._custom_dve(
                        EXP_POLY3,
                        out=u2[:, 1024 * pair:1024 * (pair + 1)],
                        in0=sqs[pair][:],
                        s0=float(EXP_A), s1=float(EXP_B),
                        imm2=float(EXP_C))
                nc.vector._custom_dve(
                    EXP_POW8,
                    out=em2[:].rearrange("p (a b) -> p a b", a=4),
                    in0=u2[:].rearrange("p (a b) -> p a b", a=4), in1=mb,
                    s0=float(EXP_INVC))
            else:
                e2 = work.tile([128, 2048], dt.bfloat16, tag="e", bufs=4,
                               name=f"e{qg}_{kb}")
                for pair in range(2):
                    nc.scalar.activation(e2[:, 1024 * pair:1024 * (pair + 1)],
                                         sqs[pair][:],
                                         mybir.ActivationFunctionType.Exp,
                                         scale=float(SCALE))
                nc.vector.tensor_mul(
                    em2[:].rearrange("p (a b) -> p a b", a=4),
                    e2[:].rearrange("p (a b) -> p a b", a=4), mb)
            # all 4 PV matmuls first (disjoint col groups -> concurrent),
            # then all 4 rowsum matmuls (again disjoint col groups).
            for h in range(4):
                nc.tensor.matmul(
                    pv_ps[32 * h:32 * (h + 1), :],
                    v_sb[kb][:, 32 * h:32 * (h + 1)],
                    em2[:, 512 * h:512 * (h + 1)],
                    start=(kb == 0), stop=(kb == KB - 1),
                    tile_position=(0, 32 * h), skip_group_check=True)
            for h in range(4):
                nc.tensor.matmul(
                    rs_ps[32 * h:32 * h + 1, :],
                    ones_sb[:], em2[:, 512 * h:512 * (h + 1)],
                    start=(kb == 0), stop=(kb == KB - 1),
                    tile_position=(0, 32 * h), skip_group_check=True)

        def emit_tail_a(st):
            # broadcast rowsums to 128 partitions, reciprocal, scale, bias
            qg, pv_ps, recs = st["qg"], st["pv"], st["recs"]
            qsl = slice(512 * qg, 512 * (qg + 1))
            bc_ps = psq.tile([128, 1024], dt.float32, tag="sq",
                             name=f"bc{qg}")
            nc.tensor.matmul(bc_ps[:, 0:512], ind4b[:], recs[:],
                             start=True, stop=True)
            st["bc_ps"] = bc_ps
            rec128 = work.tile([128, 512], dt.float32, tag="bcs", bufs=2,
                               name=f"rec128_{qg}")
            nc.vector.reciprocal_approx_fast(out=rec128[:],
                                             in_=bc_ps[:, 0:512])
            t1 = work.tile([128, 512], dt.float32, tag="t1", bufs=2,
                           name=f"t1_{qg}")
            nc.vector.tensor_mul(t1[:], pv_ps[:], rec128[:])
            nc.vector.tensor_scalar_add(yn_sb[:, qsl], t1[:], bv_sb[:])

        def emit_tail_b(st):
            # out-projection O^T = WoE.T @ yn, copy out, DMA
            qg = st["qg"]
            qsl = slice(512 * qg, 512 * (qg + 1))
            op_ps = st["bc_ps"]
            for mt in range(2):
                osl = slice(512 * mt, 512 * (mt + 1))
                nc.tensor.matmul(op_ps[:, osl],
                                 woe_sb[:, 128 * mt:128 * (mt + 1)],
                                 yn_sb[:, qsl], start=True, stop=True)
            for mt in range(2):
                osl = slice(512 * mt, 512 * (mt + 1))
                ot = tailp.tile([128, 512], dt.bfloat16, tag=f"ot{mt}",
                                name=f"ot{qg}_{mt}")
                with nc.allow_low_precision("bf16 output partials"):
                    nc.vector.tensor_copy(ot[:], op_ps[:, osl])
                for ch in range(2):
                    csl = slice(256 * ch, 256 * (ch + 1))
                    osl2 = slice(512 * qg + 256 * ch, 512 * qg + 256 * (ch + 1))
                    nc.sync.dma_start(out=OUT[128 * mt:128 * (mt + 1), osl2],
                                      in_=ot[:, csl])

        pend = None
        for qg in range(QG):
            pv_ps = ppv.tile([128, 512], dt.float32, tag="pv",
                             name=f"pv{qg}")
            rs_ps = ppv.tile([128, 512], dt.float32, tag="rs",
                             name=f"rs{qg}")
            for kb in range(KB):
                sqs = emit_scores(qg, kb)
                if pend is not None and kb == 0:
                    emit_tail_a(pend)
                emit_softmax_pv(qg, kb, sqs, pv_ps, rs_ps)
                if pend is not None and kb == 1:
                    emit_tail_b(pend)
                    pend = None
            recs = work.tile([128, 512], dt.bfloat16, tag="recs", bufs=2,
                             name=f"recs{qg}")
            with nc.allow_low_precision("softmax rowsum bf16"):
                nc.vector.tensor_copy(recs[:], rs_ps[:])
            pend = {"qg": qg, "pv": pv_ps, "recs": recs}
        emit_tail_a(pend)
        emit_tail_b(pend)
    nc.finalize()
    return nc


_NC_CACHE = None


def kernel(x, allow_mask_bool, W_qkv, b_qkv, W_out, b_out):
    global _NC_CACHE
    x = np.asarray(x, np.float32)
    allow = np.asarray(allow_mask_bool)
    W_qkv = np.asarray(W_qkv, np.float32)
    b_qkv = np.asarray(b_qkv, np.float32)
    W_out = np.asarray(W_out, np.float32)
    b_out = np.asarray(b_out, np.float32)

    M01T = np.ascontiguousarray(allow.T).astype(BF16)
    in_maps = []
    for c in range(NCORES):
        b = c // 2
        hs = [4 * (c % 2) + i for i in range(4)]
        qcols = np.concatenate([np.arange(32 * h, 32 * h + 32) for h in hs])
        m = {
            "xT": np.ascontiguousarray(x[b].T).astype(BF16),
            "Wq": np.ascontiguousarray(W_qkv[:, qcols]).astype(BF16),
            "Wk": np.ascontiguousarray(W_qkv[:, 256 + qcols]).astype(BF16),
            "Wv": np.ascontiguousarray(W_qkv[:, 512 + qcols]).astype(BF16),
            "bq": np.ascontiguousarray(b_qkv[qcols][:, None]),
            "bk": np.ascontiguousarray(b_qkv[256 + qcols][:, None]),
            "bv": np.ascontiguousarray(b_qkv[512 + qcols][:, None]),
            "M01T": M01T,
            "WoE": np.ascontiguousarray(W_out[qcols, :]).astype(BF16),
        }
        in_maps.append(m)

    global LAST_IN_MAPS
    LAST_IN_MAPS = in_maps
    if _NC_CACHE is None:
        _NC_CACHE = build_nc()
    res = run_bass_kernel_spmd(_NC_CACHE, in_maps, core_ids=list(range(NCORES)))
    out = np.zeros((B, G, D), np.float32)
    for c in range(NCORES):
        out[c // 2] += np.asarray(res.results[c]["out"], np.float32).T
    out += b_out[None, None, :]
    return out


if __name__ == "__main__":
    rng = np.random.default_rng(0)
    ins = {
        "x": rng.standard_normal((B, G, D), dtype=np.float32),
        "allow_mask_bool": rng.random((G, G)) < 0.5,
        "W_qkv": rng.standard_normal((D, 3 * D), dtype=np.float32) * 0.06,
        "b_qkv": rng.standard_normal(3 * D).astype(np.float32) * 0.06,
        "W_out": rng.standard_normal((D, D), dtype=np.float32) * 0.06,
        "b_out": rng.standard_normal(D).astype(np.float32) * 0.06,
    }
    ins["allow_mask_bool"] |= np.eye(G, dtype=bool)
    out = kernel(**ins)
    print("kernel ran, out shape", out.shape)


# revision 36
# speedup vs baseline: 1.0041x; 1.0041x over previous
"""LocalGraphAttention TRN2 kernel: 8-core SPMD (data-parallel B x head-parallel).

Layout strategy (per core c): b = c//2, heads = 4*(c%2) + [0..3].
Everything kept transposed so the softmax key-reduction is a PE matmul:
  xT (D, G) -> QT/KT stacks (128=4h*32, G) -> S^T = K @ Q^T per head
  (4-head packed via tile_position row tiling, contraction d=32),
  exp on ScalarE (PSUM->SBUF bf16, no rowmax needed: |scores|<4),
  multiplicative 0/1 mask on VectorE (FD=1024 via partition-free
  broadcast of the shared 512-col mask block across the 2-head span),
  P@V + rowsum via col-tiled matmuls accumulating y^T (128=4h*32, G)
  and rowsums in PSUM (double-buffered per query-group), then a
  normalization + out-projection tail that reuses the pv/rs banks
  (indicator broadcast matmul, reciprocal_approx_fast, scale+bias,
  O^T = WoE.T @ yn) so it overlaps the next query-group's attention.
Host gathers: out[b] = (OT_even + OT_odd).T + b_out.
"""
import sys
import numpy as np
import ml_dtypes

sys.path.insert(0, "/opt/trn_rl_repo")

from contextlib import ExitStack

import concourse.bass as bass
import concourse.mybir as mybir
import concourse.tile as tile
from concourse import bacc
from concourse.bass_utils import run_bass_kernel_spmd
import concourse.dve_ops as dops
from concourse.dve_spec import Spec, Src0, Src1, C0, C1, C2, sq as dve_sq

# Custom DVE exp: em ∝ P3(s)^8 * mask, with P3 a monic deg-3 Horner fit of
# exp(y) on y = s*SCALE/8 (rel spread ~2e-3 over the score range; the
# constant factor cancels exactly in the softmax normalization).
# Offloads part of the softmax exp from the saturated ScalarE to VectorE.
_b1 = ((Src0 + C0) * Src0 + C1) * Src0 + C2


def _ref_poly3(in0, in1, s0, s1, imm2):
    x = in0.astype(np.float32)
    return (((x + s0) * x + s1) * x + imm2).astype(np.float32)


EXP_POLY3 = dops.DveOp(
    "EXP_POLY3M_ANT", Spec(body=_b1, reference=_ref_poly3), subdim=False,
    uops_sha={"v3": "5c7cb73d73810334", "v4": "6aaa157e27d4f5b9"})


def _ref_pow8(in0, in1, s0, s1, imm2):
    u = in0.astype(np.float32) * s0
    return (u ** 8 * in1).astype(np.float32)


EXP_POW8 = dops.DveOp(
    "EXP_POW8S_MASK_ANT",
    Spec(body=dve_sq(dve_sq(dve_sq(Src0 * C0))) * Src1, reference=_ref_pow8),
    subdim=False,
    uops_sha={"v3": "27cd9c8766500aea", "v4": "689a5e63c95e4ee9"})


def _register_dve_op(op):
    if op.name not in dops._SUB_OPCODE_FOR_NAME:
        dops.OPS.append(op)
        dops.CUSTOM_DVE_SPECS[op.name] = op.spec
        dops._SUB_OPCODE_FOR_NAME[op.name] = (
            dops._CUSTOM_DVE_ROW_BASE + len(dops.OPS) - 1)


_register_dve_op(EXP_POLY3)
_register_dve_op(EXP_POW8)

# monic deg-3 coefficients in raw-score space and the pow8 rescale
EXP_A = 139.68961777842475
EXP_B = 12472.81414063659
EXP_C = 564137.3193919723
EXP_INVC = 1.772618058804904e-06
# iterations routed to the VectorE exp path
DVE_EXP_ITERS = set()

BF16 = ml_dtypes.bfloat16
G = 2048
D = 256
NH = 8
DH = 32
B = 4
NCORES = 8
SCALE = 1.0 / np.sqrt(np.float32(DH))
KB = G // 128   # 16 key blocks
QG = G // 512   # 4 query groups


def build_nc():
    nc = bacc.Bacc("TRN2", target_bir_lowering=False, debug=False)
    dt = mybir.dt
    xT = nc.declare_dram_parameter("xT", [D, G], dt.bfloat16, isOutput=False)
    Wq = nc.declare_dram_parameter("Wq", [D, 128], dt.bfloat16, isOutput=False)
    Wk = nc.declare_dram_parameter("Wk", [D, 128], dt.bfloat16, isOutput=False)
    Wv = nc.declare_dram_parameter("Wv", [D, 128], dt.bfloat16, isOutput=False)
    bq = nc.declare_dram_parameter("bq", [128, 1], dt.float32, isOutput=False)
    bk = nc.declare_dram_parameter("bk", [128, 1], dt.float32, isOutput=False)
    bv = nc.declare_dram_parameter("bv", [128, 1], dt.float32, isOutput=False)
    M01T = nc.declare_dram_parameter("M01T", [G, G], dt.bfloat16, isOutput=False)
    WoE = nc.declare_dram_parameter("WoE", [128, D], dt.bfloat16, isOutput=False)
    OUT = nc.declare_dram_parameter("out", [D, G], dt.bfloat16, isOutput=True)

    with tile.TileContext(nc) as tc, ExitStack() as ctx:
        singles = ctx.enter_context(tc.tile_pool(name="singles", bufs=1))
        maskp = ctx.enter_context(tc.tile_pool(name="maskp", bufs=KB))
        vp = ctx.enter_context(tc.tile_pool(name="vp", bufs=KB))
        work = ctx.enter_context(tc.tile_pool(name="work", bufs=6))
        tailp = ctx.enter_context(tc.tile_pool(name="tailp", bufs=2))
        psq = ctx.enter_context(tc.tile_pool(name="psq", bufs=3, space="PSUM"))
        ppv = ctx.enter_context(tc.tile_pool(name="ppv", bufs=1, space="PSUM"))

        # ---- resident loads ----
        xt = [singles.tile([128, G], dt.bfloat16, tag="xt0", name="xt0"),
              singles.tile([128, G], dt.bfloat16, tag="xt1", name="xt1")]
        for ch in range(4):
            csl = slice(512 * ch, 512 * (ch + 1))
            for kc in range(2):
                nc.sync.dma_start(out=xt[kc][:, csl],
                                  in_=xT[128 * kc:128 * (kc + 1), csl])
        wght = {}
        for name, p in (("wq", Wq), ("wk", Wk), ("wv", Wv)):
            for kc in range(2):
                t = singles.tile([128, 128], dt.bfloat16, tag=f"{name}{kc}")
                nc.sync.dma_start(out=t[:], in_=p[128 * kc:128 * (kc + 1), :])
                wght[f"{name}{kc}"] = t
        bq_sb = singles.tile([128, 1], dt.float32, tag="bq")
        nc.sync.dma_start(out=bq_sb[:], in_=bq[:])
        bk_sb = singles.tile([128, 1], dt.float32, tag="bk")
        nc.sync.dma_start(out=bk_sb[:], in_=bk[:])
        bv_sb = singles.tile([128, 1], dt.float32, tag="bv")
        nc.sync.dma_start(out=bv_sb[:], in_=bv[:])
        woe_sb = singles.tile([128, D], dt.bfloat16, tag="woe")
        nc.sync.dma_start(out=woe_sb[:], in_=WoE[:])
        m_sb = []
        for kb in range(KB):
            t = maskp.tile([128, G], dt.bfloat16, tag="mask")
            for ch in range(2):
                csl = slice(1024 * ch, 1024 * (ch + 1))
                nc.sync.dma_start(out=t[:, csl],
                                  in_=M01T[128 * kb:128 * (kb + 1), csl])
            m_sb.append(t)
        ones_sb = singles.tile([128, 1], dt.bfloat16, tag="ones")
        nc.vector.memset(ones_sb[:], 1.0)

        ind4b = singles.tile([128, 128], dt.bfloat16, tag="ind4b")
        nc.vector.memset(ind4b[:], 0.0)
        for h in range(4):
            nc.vector.memset(ind4b[32 * h:32 * h + 1, 32 * h:32 * (h + 1)],
                             1.0)

        # ---- QKV projections (kt first so scores can start earliest) ----
        qt_sb = singles.tile([128, G], dt.bfloat16, tag="qt")
        kt_sb = singles.tile([128, G], dt.bfloat16, tag="kt")
        for dst, wn, b_sb in ((kt_sb, "wk", bk_sb), (qt_sb, "wq", bq_sb)):
            for qg in range(QG):
                ps = psq.tile([128, 1024], dt.float32, tag="sq")
                sl = slice(512 * qg, 512 * (qg + 1))
                nc.tensor.matmul(ps[:, 0:512], wght[wn + "0"][:],
                                 xt[0][:, sl], start=True, stop=False)
                nc.tensor.matmul(ps[:, 0:512], wght[wn + "1"][:],
                                 xt[1][:, sl], start=False, stop=True)
                nc.vector.tensor_scalar_add(dst[:, sl], ps[:, 0:512], b_sb[:])
        v_sb = []
        for kb in range(KB):
            ps = psq.tile([128, 1024], dt.float32, tag="sq")
            sl = slice(128 * kb, 128 * (kb + 1))
            nc.tensor.matmul(ps[:, 0:128], xt[0][:, sl], wght["wv0"][:],
                             start=True, stop=False)
            nc.tensor.matmul(ps[:, 0:128], xt[1][:, sl], wght["wv1"][:],
                             start=False, stop=True)
            t = vp.tile([128, 128], dt.bfloat16, tag="v")
            nc.vector.tensor_copy(t[:], ps[:, 0:128])
            v_sb.append(t)

        # ---- attention (software-pipelined per-qg tail) ----
        yn_sb = singles.tile([128, G], dt.bfloat16, tag="yn")

        def emit_scores(qg, kb):
            qsl = slice(512 * qg, 512 * (qg + 1))
            sqs = [psq.tile([128, 1024], dt.float32, tag="sq", name=f"sq{qg}_{kb}_{p}")
                   for p in range(2)]
            for pair in range(2):
                for j in range(2):
                    h = 2 * pair + j
                    hsl = slice(32 * h, 32 * (h + 1))
                    nc.tensor.matmul(
                        sqs[pair][:, 512 * j:512 * (j + 1)],
                        kt_sb[hsl, 128 * kb:128 * (kb + 1)],
                        qt_sb[hsl, qsl],
                        start=True, stop=True, tile_position=(32 * h, 0))
            return sqs

        def emit_softmax_pv(qg, kb, sqs, pv_ps, rs_ps):
            qsl = slice(512 * qg, 512 * (qg + 1))
            mb = m_sb[kb][:, qsl].unsqueeze(1).broadcast_to([128, 4, 512])
            em2 = work.tile([128, 2048], dt.bfloat16, tag="em", bufs=4,
                            name=f"em{qg}_{kb}")
            if (qg, kb) in DVE_EXP_ITERS:
                u2 = work.tile([128, 2048], dt.float32, tag="u", bufs=2,
                               name=f"u{qg}_{kb}")
                for pair in range(2):
                    nc.vector._custom_dve(
                        EXP_POLY3,
                        out=u2[:, 1024 * pair:1024 * (pair + 1)],
                        in0=sqs[pair][:],
                        s0=float(EXP_A), s1=float(EXP_B),
                        imm2=float(EXP_C))
                nc.vector._custom_dve(
                    EXP_POW8,
                    out=em2[:].rearrange("p (a b) -> p a b", a=4),
                    in0=u2[:].rearrange("p (a b) -> p a b", a=4), in1=mb,
                    s0=float(EXP_INVC))
            else:
                e2 = work.tile([128, 2048], dt.bfloat16, tag="e", bufs=4,
                               name=f"e{qg}_{kb}")
                for pair in range(2):
                    nc.scalar.activation(e2[:, 1024 * pair:1024 * (pair + 1)],
                                         sqs[pair][:],
                                         mybir.ActivationFunctionType.Exp,
                                         scale=float(SCALE))
                nc.vector.tensor_mul(
                    em2[:].rearrange("p (a b) -> p a b", a=4),
                    e2[:].rearrange("p (a b) -> p a b", a=4), mb)
            # all 4 PV matmuls first (disjoint col groups -> concurrent),
            # then all 4 rowsum matmuls (again disjoint col groups).
            for h in range(4):
                nc.tensor.matmul(
                    pv_ps[32 * h:32 * (h + 1), :],
                    v_sb[kb][:, 32 * h:32 * (h + 1)],
                    em2[:, 512 * h:512 * (h + 1)],
                    start=(kb == 0), stop=(kb == KB - 1),
                    tile_position=(0, 32 * h), skip_group_check=True)
            for h in range(4):
                nc.tensor.matmul(
                    rs_ps[32 * h:32 * h + 1, :],
                    ones_sb[:], em2[:, 512 * h:512 * (h + 1)],
                    start=(kb == 0), stop=(kb == KB - 1),
                    tile_position=(0, 32 * h), skip_group_check=True)

        def emit_tail_a(st):
            # broadcast rowsums to 128 partitions, reciprocal, scale, bias
            qg, pv_ps, recs = st["qg"], st["pv"], st["recs"]
            qsl = slice(512 * qg, 512 * (qg + 1))
            bc_ps = psq.tile([128, 1024], dt.float32, tag="sq",
                             name=f"bc{qg}")
            nc.tensor.matmul(bc_ps[:, 0:512], ind4b[:], recs[:],
                             start=True, stop=True)
            st["bc_ps"] = bc_ps
            rec128 = work.tile([128, 512], dt.float32, tag="bcs", bufs=2,
                               name=f"rec128_{qg}")
            nc.vector.reciprocal_approx_fast(out=rec128[:],
                                             in_=bc_ps[:, 0:512])
            t1 = work.tile([128, 512], dt.float32, tag="t1", bufs=2,
                           name=f"t1_{qg}")
            nc.vector.tensor_mul(t1[:], pv_ps[:], rec128[:])
            nc.vector.tensor_scalar_add(yn_sb[:, qsl], t1[:], bv_sb[:])

        def emit_tail_b(st):
            # out-projection O^T = WoE.T @ yn, copy out, DMA
            qg = st["qg"]
            qsl = slice(512 * qg, 512 * (qg + 1))
            op_ps = st["bc_ps"]
            for mt in range(2):
                osl = slice(512 * mt, 512 * (mt + 1))
                nc.tensor.matmul(op_ps[:, osl],
                                 woe_sb[:, 128 * mt:128 * (mt + 1)],
                                 yn_sb[:, qsl], start=True, stop=True)
            for mt in range(2):
                osl = slice(512 * mt, 512 * (mt + 1))
                ot = tailp.tile([128, 512], dt.bfloat16, tag=f"ot{mt}",
                                name=f"ot{qg}_{mt}")
                with nc.allow_low_precision("bf16 output partials"):
                    nc.vector.tensor_copy(ot[:], op_ps[:, osl])
                for ch in range(2):
                    csl = slice(256 * ch, 256 * (ch + 1))
                    osl2 = slice(512 * qg + 256 * ch, 512 * qg + 256 * (ch + 1))
                    nc.sync.dma_start(out=OUT[128 * mt:128 * (mt + 1), osl2],
                                      in_=ot[:, csl])

        pend = None
        for qg in range(QG):
            pv_ps = ppv.tile([128, 512], dt.float32, tag="pv",
                             name=f"pv{qg}")
            rs_ps = ppv.tile([128, 512], dt.float32, tag="rs",
                             name=f"rs{qg}")
            for kb in range(KB):
                sqs = emit_scores(qg, kb)
                if pend is not None and kb == 0:
                    emit_tail_a(pend)
                emit_softmax_pv(qg, kb, sqs, pv_ps, rs_ps)
                if pend is not None and kb == 1:
                    emit_tail_b(pend)
                    pend = None
            recs = work.tile([128, 512], dt.bfloat16, tag="recs", bufs=2,
                             name=f"recs{qg}")
            with nc.allow_low_precision("softmax rowsum bf16"):
                nc.vector.tensor_copy(recs[:], rs_ps[:])
            pend = {"qg": qg, "pv": pv_ps, "recs": recs}
        emit_tail_a(pend)
        emit_tail_b(pend)
    nc.finalize()
    return nc


_NC_CACHE = None


def kernel(x, allow_mask_bool, W_qkv, b_qkv, W_out, b_out):
    global _NC_CACHE
    x = np.asarray(x, np.float32)
    allow = np.asarray(allow_mask_bool)
    W_qkv = np.asarray(W_qkv, np.float32)
    b_qkv = np.asarray(b_qkv, np.float32)
    W_out = np.asarray(W_out, np.float32)
    b_out = np.asarray(b_out, np.float32)

    M01T = np.ascontiguousarray(allow.T).astype(BF16)
    in_maps = []
    for c in range(NCORES):
        b = c // 2
        hs = [4 * (c % 2) + i for i in range(4)]
        qcols = np.concatenate([np.arange(32 * h, 32 * h + 32) for h in hs])
        m = {
            "xT": np.ascontiguousarray(x[b].T).astype(BF16),
            "Wq": np.ascontiguousarray(W_qkv[:, qcols]).astype(BF16),
            "Wk": np.ascontiguousarray(W_qkv[:, 256 + qcols]).astype(BF16),
            "Wv": np.ascontiguousarray(W_qkv[:, 512 + qcols]).astype(BF16),
            "bq": np.ascontiguousarray(b_qkv[qcols][:, None]),
            "bk": np.ascontiguousarray(b_qkv[256 + qcols][:, None]),
            "bv": np.ascontiguousarray(b_qkv[512 + qcols][:, None]),
            "M01T": M01T,
            "WoE": np.ascontiguousarray(W_out[qcols, :]).astype(BF16),
        }
        in_maps.append(m)

    global LAST_IN_MAPS
    LAST_IN_MAPS = in_maps
    if _NC_CACHE is None:
        _NC_CACHE = build_nc()
    res = run_bass_kernel_spmd(_NC_CACHE, in_maps, core_ids=list(range(NCORES)))
    out = np.zeros((B, G, D), np.float32)
    for c in range(NCORES):
        out[c // 2] += np.asarray(res.results[c]["out"], np.float32).T
    out += b_out[None, None, :]
    return out


if __name__ == "__main__":
    rng = np.random.default_rng(0)
    ins = {
        "x": rng.standard_normal((B, G, D), dtype=np.float32),
        "allow_mask_bool": rng.random((G, G)) < 0.5,
        "W_qkv": rng.standard_normal((D, 3 * D), dtype=np.float32) * 0.06,
        "b_qkv": rng.standard_normal(3 * D).astype(np.float32) * 0.06,
        "W_out": rng.standard_normal((D, D), dtype=np.float32) * 0.06,
        "b_out": rng.standard_normal(D).astype(np.float32) * 0.06,
    }
    ins["allow_mask_bool"] |= np.eye(G, dtype=bool)
    out = kernel(**ins)
    print("kernel ran, out shape", out.shape)
